# revision 16
# baseline (speedup 1.0000x reference)
"""Trainium2 Bass kernel for nn_EnhancedGCN42 (4-layer GCN + MLP classifier).

Strategy (8 NeuronCores, SPMD single NEFF):
  - Nodes dst-sharded: device d owns dst nodes [d*12500, (d+1)*12500).
  - A-hat = D^-1/2 (A+I) D^-1/2 factorized: tables store dis*h rows (bf16,
    256B rows); aggregation output scaled by dis_dst.
  - Per layer: per-edge rows gathered via dma_gather (4 SWDGE queues),
    aggregated per 128-dst tile by matmul with an on-chip-built one-hot
    selection matrix (is_equal against iota).
  - Dense W / BN / ReLU fused per dst-tile in transposed layout; BN and
    classifier BN folded on host into per-feature scale/bias.
  - AllGather (collective) replicates next layer's table between phases.

kernel(**inputs) -> [100000, 2] float32.
"""
import hashlib
import numpy as np
import ml_dtypes

import concourse.bacc as bacc
import concourse.bass as bass
import concourse.mybir as mybir
import concourse.tile as tile
from concourse.masks import make_identity
from concourse.bass_utils import run_bass_kernel_spmd

f32 = mybir.dt.float32
bf16 = mybir.dt.bfloat16
i16 = mybir.dt.int16
i32 = mybir.dt.int32
nbf16 = ml_dtypes.bfloat16

P = 128
NDEV = 8
NR = 4           # src index ranges (int16 limit)
EPS = 1e-5
WTAB = 128       # table row = 128 cols bf16 = 256B


def _prep(x, edge_index, params, N):
    """Host preprocessing: graph partition + folded constants. Returns meta dict."""
    SHARD = N // NDEV
    TSHARD = ((SHARD + P - 1) // P) * P
    NT = TSHARD // P
    TROWS = TSHARD * NDEV
    RNGW = TROWS // NR
    assert RNGW <= 32768 and RNGW % P == 0

    ei = edge_index.astype(np.int64)
    loop = np.arange(N, dtype=np.int64)
    src = np.concatenate([ei[0], loop])
    dst = np.concatenate([ei[1], loop])
    deg = np.bincount(dst, minlength=N).astype(np.float32)
    dis = (1.0 / np.sqrt(deg)).astype(np.float32)

    shard_of = src // SHARD
    psrc = shard_of * TSHARD + (src - shard_of * SHARD)

    # per-device pane structure
    counts = np.zeros((NDEV, NT, NR), dtype=np.int64)
    dev_edges = []
    for d in range(NDEV):
        m = (dst >= d * SHARD) & (dst < (d + 1) * SHARD)
        es = psrc[m]
        el = dst[m] - d * SHARD
        t_id = el >> 7
        r_id = es // RNGW
        order = np.lexsort((r_id, t_id))
        es, el, t_id, r_id = es[order], el[order], t_id[order], r_id[order]
        np.add.at(counts[d], (t_id, r_id), 1)
        dev_edges.append((es, el, t_id, r_id))

    nch = np.ceil(counts.max(axis=0) / P).astype(np.int64)  # [NT, NR] common
    tot_ch = int(nch.sum())
    TOT = tot_ch * P

    # block-merged gather layout: order = (block, r, t-in-block); each (t, r)
    # group 128-padded so chunk columns stay tile-pure within the big gather.
    BLK = 4
    n_blk = (NT + BLK - 1) // BLK
    pane_off = np.zeros((NT, NR), dtype=np.int64)   # row offset of (t, r) group
    blk_off = np.zeros((n_blk, NR), dtype=np.int64)  # row offset of gather (b, r)
    blk_rows = np.zeros((n_blk, NR), dtype=np.int64)
    acc = 0
    for b in range(n_blk):
        for r in range(NR):
            blk_off[b, r] = acc
            for t in range(b * BLK, min((b + 1) * BLK, NT)):
                pane_off[t, r] = acc
                acc += nch[t, r] * P
            blk_rows[b, r] = acc - blk_off[b, r]
    assert acc == TOT

    idx_w = np.zeros((NDEV, P, TOT // 16), dtype=np.int16)
    dstl_w = np.full((NDEV, P, tot_ch), 255.0, dtype=np.float32)
    for d in range(NDEV):
        es, el, t_id, r_id = dev_edges[d]
        IDX = np.zeros(TOT, dtype=np.int16)
        DSTL = np.full(TOT, 255.0, dtype=np.float32)
        # per-(t,r) source position in the (t-major, r-minor) sorted edge list
        pos = 0
        for t in range(NT):
            for r in range(NR):
                c = counts[d, t, r]
                o = pane_off[t, r]
                osrc = np.argsort(es[pos:pos + c], kind="stable")  # ascending src: DMA locality
                IDX[o:o + c] = (es[pos:pos + c][osrc] - r * RNGW).astype(np.int16)
                # idx pad default 0 (valid row of range) via zeros init
                DSTL[o:o + c] = (el[pos:pos + c][osrc] - t * P).astype(np.float32)
                pos += c
        idx_w[d] = np.tile(IDX.reshape(-1, 16).T, (8, 1))
        dstl_w[d] = DSTL.reshape(tot_ch, P).T

    dis_pad = np.zeros(TROWS, dtype=np.float32)
    for s in range(NDEV):
        dis_pad[s * TSHARD:s * TSHARD + SHARD] = dis[s * SHARD:(s + 1) * SHARD]
    dis_t = np.stack([
        dis_pad[d * TSHARD:(d + 1) * TSHARD].reshape(NT, P).T for d in range(NDEV)
    ])  # [NDEV, P, NT]

    # x-tilde table (bf16, padded cols)
    xt = np.zeros((TROWS, WTAB), dtype=nbf16)
    v = (dis[:, None] * x).astype(nbf16)
    for s in range(NDEV):
        xt[s * TSHARD:s * TSHARD + SHARD, :x.shape[1]] = v[s * SHARD:(s + 1) * SHARD]

    def fold(g, be, rm, rv, b):
        k = (1.0 / np.sqrt(rv + EPS)).astype(np.float32)
        s = g * k
        t = (b - rm) * s + be
        return s.astype(np.float32), t.astype(np.float32)

    s1, t1 = fold(params["g1"], params["be1"], params["rm1"], params["rv1"], params["b1"])
    s2, t2 = fold(params["g2"], params["be2"], params["rm2"], params["rv2"], params["b2"])
    s3, t3 = fold(params["g3"], params["be3"], params["rm3"], params["rv3"], params["b3"])
    s4, t4 = fold(params["g4"], params["be4"], params["rm4"], params["rv4"], params["b4"])
    zk = (1.0 / np.sqrt(params["crv1"] + EPS)).astype(np.float32)
    cs1 = params["cg1"] * zk
    ct1 = -params["crm1"] * cs1 + params["cbe1"]
    zk = (1.0 / np.sqrt(params["crv2"] + EPS)).astype(np.float32)
    cs2 = params["cg2"] * zk
    ct2 = -params["crm2"] * cs2 + params["cbe2"]
    cW2p = (cs1[:, None] * params["cW2"]).astype(np.float32)
    cb2p = (ct1 @ params["cW2"] + params["cb2"]).astype(np.float32)
    cW3p = (cs2[:, None] * params["cW3"]).astype(np.float32)
    cb3p = (ct2 @ params["cW3"] + params["cb3"]).astype(np.float32)

    vecs = np.zeros((P, 13), dtype=np.float32)
    vecs[:, 0], vecs[:, 1] = s1, t1
    vecs[:, 2], vecs[:, 3] = s2[:128], t2[:128]
    vecs[:, 4], vecs[:, 5] = s2[128:], t2[128:]
    vecs[:, 6], vecs[:, 7] = s3, t3
    vecs[:64, 8], vecs[:64, 9] = s4, t4
    vecs[:64, 10] = params["cb1"]
    vecs[:32, 11] = cb2p
    vecs[:2, 12] = cb3p

    return dict(
        N=N, SHARD=SHARD, TSHARD=TSHARD, NT=NT, TROWS=TROWS, RNGW=RNGW,
        nch=nch, tot_ch=tot_ch, TOT=TOT, pane_off=pane_off,
        BLK=BLK, n_blk=n_blk, blk_off=blk_off, blk_rows=blk_rows,
        idx_w=idx_w, dstl_w=dstl_w, dis_t=dis_t, xt=xt, vecs=vecs,
        W1=params["W1"].astype(np.float32), W2=params["W2"].astype(np.float32),
        W3=np.concatenate([params["W3"][:128], params["W3"][128:]], axis=1).astype(np.float32),
        W4=params["W4"].astype(np.float32),
        cW1=params["cW1"].astype(np.float32), cW2p=cW2p, cW3p=cW3p,
        d_in=x.shape[1],
    )


def _build(meta):
    """Build the Bass program (same for all cores)."""
    NT, TSHARD, TROWS, RNGW = meta["NT"], meta["TSHARD"], meta["TROWS"], meta["RNGW"]
    TOT, tot_ch = meta["TOT"], meta["tot_ch"]
    nch, pane_off = meta["nch"], meta["pane_off"]
    BLK, n_blk, blk_off, blk_rows = meta["BLK"], meta["n_blk"], meta["blk_off"], meta["blk_rows"]
    D_IN = meta["d_in"]

    nc = bacc.Bacc(None, target_bir_lowering=False, num_swdge_queues=4)
    t_xt = nc.dram_tensor("xt", [TROWS, WTAB], bf16, kind="ExternalInput")
    t_idx = nc.dram_tensor("idx", [P, TOT // 16], i16, kind="ExternalInput")
    t_dstl = nc.dram_tensor("dstl", [P, tot_ch], f32, kind="ExternalInput")
    t_dis = nc.dram_tensor("dis", [P, NT], f32, kind="ExternalInput")
    t_vecs = nc.dram_tensor("vecs", [P, 13], f32, kind="ExternalInput")
    t_W1 = nc.dram_tensor("W1", [D_IN, 128], f32, kind="ExternalInput")
    t_W2 = nc.dram_tensor("W2", [128, 256], f32, kind="ExternalInput")
    t_W3 = nc.dram_tensor("W3", [128, 256], f32, kind="ExternalInput")  # packed K-halves
    t_W4 = nc.dram_tensor("W4", [128, 64], f32, kind="ExternalInput")
    t_cW1 = nc.dram_tensor("cW1", [64, 64], f32, kind="ExternalInput")
    t_cW2 = nc.dram_tensor("cW2p", [64, 32], f32, kind="ExternalInput")
    t_cW3 = nc.dram_tensor("cW3p", [32, 2], f32, kind="ExternalInput")
    t_out = nc.dram_tensor("outT", [2, TSHARD], f32, kind="ExternalOutput")

    cc_in = [nc.dram_tensor(f"cc_in{i}", [TSHARD, WTAB], bf16) for i in range(3)]
    tabs = [nc.dram_tensor(f"tab{i}", [TROWS, WTAB], bf16, addr_space="Shared")
            for i in range(3)]

    qctr = [0]

    def qrr():
        qctr[0] = (qctr[0] + 1) % 4
        return qctr[0]

    with tile.TileContext(nc) as tc:
        with (
            tc.tile_pool(name="const", bufs=1) as cpool,
            tc.tile_pool(name="gp", bufs=8) as gpool,
            tc.tile_pool(name="sp", bufs=6) as spool,
            tc.tile_pool(name="pagg", bufs=2, space="PSUM") as pagg,
            tc.tile_pool(name="paux", bufs=3, space="PSUM") as paux,
            tc.tile_pool(name="ep", bufs=3) as ep,
        ):
            # ---- constants
            idx_sb = cpool.tile([P, TOT // 16], i16)
            nc.sync.dma_start(out=idx_sb[:], in_=t_idx[:])
            dstl_sb = cpool.tile([P, tot_ch], f32)
            nc.sync.dma_start(out=dstl_sb[:], in_=t_dstl[:])
            dis_sb = cpool.tile([P, NT], f32)
            nc.sync.dma_start(out=dis_sb[:], in_=t_dis[:])
            vecs_sb = cpool.tile([P, 13], f32)
            nc.sync.dma_start(out=vecs_sb[:], in_=t_vecs[:])
            W1_sb = cpool.tile([D_IN, 128], f32)
            nc.sync.dma_start(out=W1_sb[:], in_=t_W1[:])
            W2_sb = cpool.tile([128, 256], f32)
            nc.sync.dma_start(out=W2_sb[:], in_=t_W2[:])
            W3_sb = cpool.tile([128, 256], f32)  # cols [h*128:(h+1)*128] = W3[h*128:(h+1)*128, :]
            nc.sync.dma_start(out=W3_sb[:], in_=t_W3[:])
            W4_sb = cpool.tile([128, 64], f32)
            nc.sync.dma_start(out=W4_sb[:], in_=t_W4[:])
            cW1_sb = cpool.tile([64, 64], f32)
            nc.sync.dma_start(out=cW1_sb[:], in_=t_cW1[:])
            cW2_sb = cpool.tile([64, 32], f32)
            nc.sync.dma_start(out=cW2_sb[:], in_=t_cW2[:])
            cW3_sb = cpool.tile([32, 2], f32)
            nc.sync.dma_start(out=cW3_sb[:], in_=t_cW3[:])
            ident = cpool.tile([P, P], f32)
            make_identity(nc, ident[:])
            KMAX = int(nch.max())
            iota_i = cpool.tile([P, KMAX, P], i32)
            nc.gpsimd.iota(iota_i[:], pattern=[[0, KMAX], [1, P]], base=0,
                           channel_multiplier=0)
            iota_bf = cpool.tile([P, KMAX, P], bf16)
            nc.vector.tensor_copy(out=iota_bf[:], in_=iota_i[:])

            AluEq = mybir.AluOpType.is_equal
            ACT = mybir.ActivationFunctionType

            def transpose_f32(src_sb, pdim, fdim):
                """[pdim, fdim] f32 sbuf -> [fdim, pdim] f32 sbuf (PE transpose)."""
                tp = paux.tile([fdim, pdim], f32, tag="mm")
                nc.tensor.transpose(tp[:], src_sb[:], ident[:pdim, :pdim])
                out = ep.tile([fdim, pdim], f32, tag="tps")
                nc.vector.tensor_copy(out=out[:], in_=tp[:])
                return out

            def phase(table_handle, w, epilogue):
                """Block-merged gathers + per-tile S-matmul aggregation."""
                for b in range(n_blk):
                    tiles = range(b * BLK, min((b + 1) * BLK, NT))
                    gt = {}
                    for r in range(NR):
                        rows = int(blk_rows[b, r])
                        if rows == 0:
                            continue
                        g = gpool.tile([P, rows // P, WTAB], bf16, tag="g")
                        off = int(blk_off[b, r])
                        nc.gpsimd.dma_gather(
                            out_ap=g[:],
                            in_ap=table_handle[r * RNGW:(r + 1) * RNGW, :],
                            idxs_ap=idx_sb[:, off // 16:(off + rows) // 16],
                            num_idxs=rows,
                            num_idxs_reg=rows,
                            elem_size=WTAB,
                            single_packet=False,
                            queue_num=qrr(),
                        )
                        gt[r] = g
                    for t in tiles:
                        panes = [(r, int(nch[t, r])) for r in range(NR) if nch[t, r] > 0]
                        spt = {}
                        for r, ch in panes:
                            gc0 = int(pane_off[t, r]) // P
                            sP = spool.tile([P, ch, P], bf16, tag="s")
                            nc.vector.tensor_tensor(
                                out=sP[:],
                                in0=dstl_sb[:, gc0:gc0 + ch].to_broadcast([P, ch, P]),
                                in1=iota_bf[:, :ch, :],
                                op=AluEq,
                            )
                            spt[r] = sP
                        nchunk = sum(ch for _, ch in panes)
                        ps = pagg.tile([P, w], f32, tag="pagg")
                        k = 0
                        for r, ch in panes:
                            col0 = (int(pane_off[t, r]) - int(blk_off[b, r])) // P
                            for c in range(ch):
                                nc.tensor.matmul(
                                    ps[:], lhsT=spt[r][:, c, :], rhs=gt[r][:, col0 + c, :w],
                                    start=(k == 0), stop=(k == nchunk - 1),
                                )
                                k += 1
                        epilogue(t, ps)

            # ================= Phase 1: L1 =================
            def ep1(t, ps):
                a = ep.tile([P, D_IN], f32, tag="a1")
                nc.scalar.activation(a[:], ps[:], ACT.Copy, scale=dis_sb[:, t:t + 1])
                aT = transpose_f32(a, P, D_IN)
                hps = paux.tile([128, P], f32, tag="mm")
                nc.tensor.matmul(hps[:], lhsT=W1_sb[:], rhs=aT[:], start=True, stop=True)
                hT = ep.tile([128, P], f32, tag="h1T")
                nc.scalar.activation(hT[:], hps[:], ACT.Relu,
                                     bias=vecs_sb[:, 1:2], scale=vecs_sb[:, 0:1])
                hp = paux.tile([P, 128], f32, tag="mm")
                nc.tensor.transpose(hp[:], hT[:], ident[:])
                hb = ep.tile([P, WTAB], bf16, tag="h1b")
                nc.scalar.activation(hb[:], hp[:], ACT.Copy, scale=dis_sb[:, t:t + 1])
                nc.sync.dma_start(out=cc_in[0][t * P:(t + 1) * P, :], in_=hb[:])

            phase(t_xt, D_IN, ep1)
            nc.gpsimd.collective_compute(
                "AllGather", mybir.AluOpType.bypass,
                replica_groups=[list(range(NDEV))],
                ins=[cc_in[0][:]], outs=[tabs[0][:]],
            )

            # ================= Phase 2: L2 + dense L3 =================
            def ep2(t, ps):
                a = ep.tile([P, 128], f32, tag="a2")
                nc.scalar.activation(a[:], ps[:], ACT.Copy, scale=dis_sb[:, t:t + 1])
                aT = transpose_f32(a, P, 128)
                y3ps = paux.tile([128, P], f32, tag="acc")
                for h in range(2):
                    hps = paux.tile([128, P], f32, tag="mm")
                    nc.tensor.matmul(hps[:], lhsT=W2_sb[:, h * 128:(h + 1) * 128],
                                     rhs=aT[:], start=True, stop=True)
                    hT = ep.tile([128, P], f32, tag="h2T")
                    nc.scalar.activation(hT[:], hps[:], ACT.Relu,
                                         bias=vecs_sb[:, 3 + 2 * h:4 + 2 * h],
                                         scale=vecs_sb[:, 2 + 2 * h:3 + 2 * h])
                    nc.tensor.matmul(y3ps[:], lhsT=W3_sb[:, h * 128:(h + 1) * 128],
                                     rhs=hT[:], start=(h == 0), stop=(h == 1))
                y3T = ep.tile([128, P], f32, tag="y3T")
                nc.vector.tensor_copy(out=y3T[:], in_=y3ps[:])
                y3p = paux.tile([P, 128], f32, tag="mm")
                nc.tensor.transpose(y3p[:], y3T[:], ident[:])
                y3b = ep.tile([P, WTAB], bf16, tag="y3b")
                nc.scalar.activation(y3b[:], y3p[:], ACT.Copy, scale=dis_sb[:, t:t + 1])
                nc.sync.dma_start(out=cc_in[1][t * P:(t + 1) * P, :], in_=y3b[:])

            phase(tabs[0], 128, ep2)
            nc.gpsimd.collective_compute(
                "AllGather", mybir.AluOpType.bypass,
                replica_groups=[list(range(NDEV))],
                ins=[cc_in[1][:]], outs=[tabs[1][:]],
            )

            # ================= Phase 3: L3 agg + dense L4 =================
            def ep3(t, ps):
                z = ep.tile([P, 128], f32, tag="z3")
                nc.scalar.activation(z[:], ps[:], ACT.Copy, scale=dis_sb[:, t:t + 1])
                zT = transpose_f32(z, P, 128)
                h3T = ep.tile([128, P], f32, tag="h3T")
                nc.scalar.activation(h3T[:], zT[:], ACT.Relu,
                                     bias=vecs_sb[:, 7:8], scale=vecs_sb[:, 6:7])
                y4ps = paux.tile([64, P], f32, tag="mm")
                nc.tensor.matmul(y4ps[:], lhsT=W4_sb[:], rhs=h3T[:], start=True, stop=True)
                y4T = ep.tile([64, P], f32, tag="y4T")
                nc.vector.tensor_copy(out=y4T[:], in_=y4ps[:])
                y4p = paux.tile([P, 64], f32, tag="mm")
                nc.tensor.transpose(y4p[:], y4T[:], ident[:64, :64])
                y4b = ep.tile([P, WTAB], bf16, tag="y4b")
                nc.vector.memset(y4b[:, 64:], 0)
                nc.scalar.activation(y4b[:, :64], y4p[:], ACT.Copy,
                                     scale=dis_sb[:, t:t + 1])
                nc.sync.dma_start(out=cc_in[2][t * P:(t + 1) * P, :], in_=y4b[:])

            phase(tabs[1], 128, ep3)
            nc.gpsimd.collective_compute(
                "AllGather", mybir.AluOpType.bypass,
                replica_groups=[list(range(NDEV))],
                ins=[cc_in[2][:]], outs=[tabs[2][:]],
            )

            # ================= Phase 4: L4 agg + classifier =================
            def ep4(t, ps):
                z = ep.tile([P, 64], f32, tag="z4")
                nc.scalar.activation(z[:], ps[:], ACT.Copy, scale=dis_sb[:, t:t + 1])
                zT = transpose_f32(z, P, 64)
                h4T = ep.tile([64, P], f32, tag="h4T")
                nc.scalar.activation(h4T[:], zT[:], ACT.Relu,
                                     bias=vecs_sb[:64, 9:10], scale=vecs_sb[:64, 8:9])
                u1ps = paux.tile([64, P], f32, tag="mm")
                nc.tensor.matmul(u1ps[:], lhsT=cW1_sb[:], rhs=h4T[:], start=True, stop=True)
                u1T = ep.tile([64, P], f32, tag="u1T")
                nc.scalar.activation(u1T[:], u1ps[:], ACT.Relu, bias=vecs_sb[:64, 10:11])
                u2ps = paux.tile([32, P], f32, tag="mm")
                nc.tensor.matmul(u2ps[:], lhsT=cW2_sb[:], rhs=u1T[:], start=True, stop=True)
                u2T = ep.tile([32, P], f32, tag="u2T")
                nc.scalar.activation(u2T[:], u2ps[:], ACT.Relu, bias=vecs_sb[:32, 11:12])
                ops_ = paux.tile([2, P], f32, tag="mm")
                nc.tensor.matmul(ops_[:], lhsT=cW3_sb[:], rhs=u2T[:], start=True, stop=True)
                oT = ep.tile([2, P], f32, tag="oT")
                nc.scalar.activation(oT[:], ops_[:], ACT.Identity, bias=vecs_sb[:2, 12:13])
                nc.sync.dma_start(out=t_out[:, t * P:(t + 1) * P], in_=oT[:])

            phase(tabs[2], 64, ep4)

    nc.finalize()
    return nc


_CACHE = {}


def kernel(**inputs):
    x = np.asarray(inputs["x"], dtype=np.float32)
    edge_index = np.asarray(inputs["edge_index"])
    N = x.shape[0]
    key = hashlib.sha256(edge_index.tobytes()).hexdigest()[:16] + f"_{N}_{x.shape[1]}"
    if key not in _CACHE:
        meta = _prep(x, edge_index, inputs, N)
        nc = _build(meta)
        _CACHE[key] = (meta, nc)
    else:
        meta, nc = _CACHE[key]
        # x may differ between calls with same graph: recompute xt
        meta = dict(meta)
        m2 = _prep(x, edge_index, inputs, N)
        meta["xt"] = m2["xt"]
        meta.update({k: m2[k] for k in ("vecs", "W1", "W2", "W3", "W4", "cW1", "cW2p", "cW3p", "dis_t")})

    in_maps = []
    for d in range(NDEV):
        in_maps.append({
            "xt": meta["xt"],
            "idx": meta["idx_w"][d],
            "dstl": meta["dstl_w"][d],
            "dis": meta["dis_t"][d],
            "vecs": meta["vecs"],
            "W1": meta["W1"], "W2": meta["W2"], "W3": meta["W3"], "W4": meta["W4"],
            "cW1": meta["cW1"], "cW2p": meta["cW2p"], "cW3p": meta["cW3p"],
        })
    res = run_bass_kernel_spmd(nc, in_maps, core_ids=list(range(NDEV)), trace=False)
    SHARD = meta["SHARD"]
    out = np.empty((N, 2), dtype=np.float32)
    for d in range(NDEV):
        out[d * SHARD:(d + 1) * SHARD] = res.results[d]["outT"][:, :SHARD].T
    return out
